# revision 30
# baseline (speedup 1.0000x reference)
"""Trainium2 Bass kernel for nn_ConstellationRelay.

Computation (per token, D=1024, A=16 anchors, C=8 comps, dc=64):
  h   = l2norm(layernorm(x; ln_g, ln_b))
  tri = 1 - h @ l2norm(anchors).T                       (N, 16)
  u   = relu(einsum('nak,kae->nke', tri_g, W1) + b1)^2  (N, 8, 128)
  y   = layernorm_c(u @ W2 + b2; cg, cb)                (N, 8, 64)
  out = x + sigmoid(gate) * (y.flat @ Wp + bp)

Strategy: pure data-parallel over batch (one of 8 NeuronCores per batch row).
On-device fast path requires ln_g==1, ln_b==0 (always true for this problem's
setup_inputs); every other parameter is handled generally via host-side
folding:
  * h = (x - mu)/sqrt(1024*var)  -- eps cancels exactly through the l2norm
  * tri/W1 stage folded into two small matmuls; biasu (sum_m W1exp + b1) is
    folded into the expand matmul via a constant-1 row in a0, so squared-ReLU
    is a single (max 0) * self op
  * comp-LN mean-subtraction folded into centered W2/b2 (host)
  * cg, cb, bp, sigmoid(gate) folded into Wp/const (host); Wp is fp8 with a
    power-of-2 scale SP compensated in the residual add
  * proj and variance matmuls run in fp8 DoubleRow mode (2 k-chunks/instr)
Layout: token-major for stats/residual, feature-major (via DMA-transpose of
bf16 h) for all matmuls; proj matmul operand-swapped so the residual add
lands token-major in PSUM.
"""

import functools
import os
import sys

import numpy as np

for _p in ("/opt/trn_rl_repo",):
    if _p not in sys.path and os.path.isdir(_p):
        sys.path.insert(0, _p)

B, S, D = 8, 4096, 1024
A, C, DC = 16, 8, 64
APC = A // C  # anchors per compartment
E2 = 2 * DC  # 128, expanded width per comp
NCORES = 8
TOK = 256  # tokens per pipeline tile
NTILE = S // TOK  # 8
NCH = TOK // 128  # 4 token chunks of 128 per tile
KD = D // 128  # 8 feature chunks
SP = 256.0  # fp8 scale on the folded projection matrix


def _np_reference(x, anchors, ln_g, ln_b, W1, b1, W2, b2, cg, cb, Wp, bp, gate):
    """Pure-numpy fallback, mirrors reference.py (used only if ln_g/ln_b
    deviate from the values this problem's setup_inputs produces)."""
    x = x.astype(np.float32)
    N = x.shape[0] * x.shape[1]
    xf = x.reshape(N, D)
    mu = xf.mean(-1, keepdims=True)
    var = ((xf - mu) ** 2).mean(-1, keepdims=True)
    h = (xf - mu) / np.sqrt(var + 1e-5) * ln_g + ln_b
    h = h / np.maximum(np.linalg.norm(h, axis=-1, keepdims=True), 1e-12)
    a = anchors / np.maximum(np.linalg.norm(anchors, axis=-1, keepdims=True), 1e-12)
    tri = 1.0 - h @ a.T
    g = tri.reshape(N, APC, C)
    u = np.einsum("nak,kae->nke", g, W1) + b1
    u = np.square(np.maximum(u, 0.0))
    y = np.einsum("nke,ked->nkd", u, W2) + b2
    muy = y.mean(-1, keepdims=True)
    vy = ((y - muy) ** 2).mean(-1, keepdims=True)
    y = (y - muy) / np.sqrt(vy + 1e-5) * cg + cb
    upd = y.reshape(N, C * DC) @ Wp + bp
    sig = 1.0 / (1.0 + np.exp(-gate))
    return (xf + sig * upd).reshape(x.shape).astype(np.float32)


@functools.lru_cache(maxsize=4)
def _build_program(n_tokens=S, use_const=False):
    """Build + schedule the single-core Bass program (same program runs SPMD
    on all 8 cores)."""
    import concourse.bacc as bacc
    import concourse.mybir as mybir
    import concourse.tile as tile

    f32 = mybir.dt.float32
    bf16 = mybir.dt.bfloat16
    fp8 = mybir.dt.float8e4
    AF = mybir.ActivationFunctionType
    OP = mybir.AluOpType
    DR = mybir.MatmulPerfMode.DoubleRow

    ntile = n_tokens // TOK

    nc = bacc.Bacc("TRN2", target_bir_lowering=False, debug=False,
                   num_devices=NCORES)

    x_d = nc.dram_tensor("x", [n_tokens, D], f32, kind="ExternalInput")
    agt_d = nc.dram_tensor("agt", [128, 4, 2, 128], fp8, kind="ExternalInput")
    w1e_d = nc.dram_tensor("w1e", [128, KD, 128], bf16, kind="ExternalInput")
    w2c_d = nc.dram_tensor("w2c", [128, C, DC], bf16, kind="ExternalInput")
    vstl_d = nc.dram_tensor("vstl", [128, 4, C], bf16, kind="ExternalInput")
    b2f_d = nc.dram_tensor("b2f", [128, 4], f32, kind="ExternalInput")
    wpf_d = nc.dram_tensor("wpf", [128, 2, 2, 2, 512], fp8,
                           kind="ExternalInput")
    sel_d = nc.dram_tensor("sel", [C, 4, 128], bf16, kind="ExternalInput")
    cvec_d = nc.dram_tensor("cvec", [1, 2, 512], bf16, kind="ExternalInput") \
        if use_const else None
    out_d = nc.dram_tensor("out", [n_tokens, D], f32, kind="ExternalOutput")

    from contextlib import ExitStack

    with tile.TileContext(nc) as tc, ExitStack() as ctx:
        ctx.enter_context(nc.allow_low_precision(
            reason="update path is damped by sigmoid(gate)~0.047; fp8/bf16 "
                   "intermediates are well within the 2e-2 tolerance"))
        pp = ctx.enter_context(tc.tile_pool(name="params", bufs=1))
        agt = pp.tile([128, 4, 2, 128], fp8)
        nc.sync.dma_start(out=agt, in_=agt_d[:, :, :, :])
        w1e = pp.tile([128, KD, 128], bf16)
        nc.sync.dma_start(out=w1e, in_=w1e_d[:, :, :])
        w2c = pp.tile([128, C, DC], bf16)
        nc.sync.dma_start(out=w2c, in_=w2c_d[:, :, :])
        vstl = pp.tile([128, 4, C], bf16)
        nc.sync.dma_start(out=vstl, in_=vstl_d[:, :, :])
        b2f = pp.tile([128, 4], f32)
        nc.sync.dma_start(out=b2f, in_=b2f_d[:, :])
        wpf = pp.tile([128, 2, 2, 2, 512], fp8)
        nc.sync.dma_start(out=wpf, in_=wpf_d[:, :, :, :, :])
        sel = pp.tile([C, 4, 128], bf16)
        nc.sync.dma_start(out=sel, in_=sel_d[:, :, :])
        if use_const:
            cvec = pp.tile([1, 2, 512], bf16)
            nc.sync.dma_start(out=cvec, in_=cvec_d[:, :, :])
            ones1 = pp.tile([1, 128], bf16)
            nc.vector.memset(ones1, 1.0)
        ctiny = pp.tile([128, 1], f32)
        nc.vector.memset(ctiny, 1e-38)
        cepsp = pp.tile([C, 1], f32)
        nc.vector.memset(cepsp, 1e-5)
        # Constant-1 routing for biasu: a0p[32r+16, :] = 1.0 via a rank-1
        # accumulation appended to the A0 matmul.
        ones512 = pp.tile([1, TOK], bf16)
        nc.vector.memset(ones512, 1.0)
        bsel = pp.tile([1, 128], bf16)
        nc.vector.memset(bsel, 0.0)
        for r in range(4):
            nc.vector.memset(bsel[0:1, 32 * r + A:32 * r + A + 1], 1.0)

        px = ctx.enter_context(tc.tile_pool(name="px", bufs=2))
        psm = ctx.enter_context(tc.tile_pool(name="psm", bufs=8))
        # PSUM pools: 3 + 2 + 2 + 1 = 8 banks exactly.
        ps_exp = ctx.enter_context(tc.tile_pool(name="ps_exp", bufs=3,
                                                space="PSUM"))
        ps_y = ctx.enter_context(tc.tile_pool(name="ps_y", bufs=2,
                                              space="PSUM"))
        ps_mm = ctx.enter_context(tc.tile_pool(name="ps_mm", bufs=2,
                                               space="PSUM"))
        ps_small = ctx.enter_context(tc.tile_pool(name="ps_small", bufs=1,
                                                  space="PSUM"))

        def stage_front_load(t):
            row0 = t * TOK
            xt = px.tile([128, NCH, D], f32, tag="xt", bufs=4, name=f"xt{t}")
            nc.sync.dma_start(
                out=xt,
                in_=x_d[row0: row0 + TOK, :].rearrange(
                    "(c p) d -> p c d", p=128))
            return xt

        def stage_front_stats(t, xt):
            """Stats + normalize + transpose."""
            hb = px.tile([128, NCH, 512], bf16, tag="hb", bufs=2,
                         name=f"hb{t}")
            mv = psm.tile([128, NCH, 2], f32, tag="mv", name=f"mv{t}")
            for cch in range(NCH):
                st = psm.tile([128, 2, 6], f32, tag="st")
                xr = xt[:, cch, :].rearrange("p (s f) -> p s f", s=2)
                nc.vector.bn_stats(out=st[:, 0, :], in_=xr[:, 0, :])
                nc.vector.bn_stats(out=st[:, 1, :], in_=xr[:, 1, :])
                nc.vector.bn_aggr(out=mv[:, cch, :], in_=st)
            # ee = 16/sqrt(D*var + tiny) = 16/||x-mu|| (fp8 h scaled by 16)
            sd = psm.tile([128, NCH], f32, tag="sd")
            nc.scalar.activation(sd, mv[:, :, 1], AF.Sqrt, bias=ctiny,
                                 scale=float(D) / 256.0)
            ee = psm.tile([128, NCH], f32, tag="ee", name=f"ee{t}")
            nc.vector.reciprocal(ee, sd)
            bh = psm.tile([128, NCH], f32, tag="bh", name=f"bh{t}")
            nc.vector.scalar_tensor_tensor(
                out=bh, in0=mv[:, :, 0], scalar=-1.0, in1=ee,
                op0=OP.mult, op1=OP.mult)
            nmu = psm.tile([128, NCH], f32, tag="nmu", name=f"nmu{t}")
            nc.vector.tensor_scalar_mul(nmu, mv[:, :, 0], -1.0)
            # hb word w packs fp8 pair (16*h[w], 16*h[w+512]); the Act/Pool
            # out AP iterates the pair halves as the outer free dim
            # (chunk 0 on Act, chunks 1-3 on Pool)
            hw0 = hb[:, 0, :].bitcast(fp8).rearrange("p (w i) -> p i w", i=2)
            nc.scalar.activation(hw0, xt[:, 0, :], AF.Identity,
                                 bias=bh[:, 0:1], scale=ee[:, 0:1])
            for cch in range(1, NCH):
                hwc = hb[:, cch, :].bitcast(fp8).rearrange(
                    "p (w i) -> p i w", i=2)
                nc.gpsimd.tensor_scalar(
                    out=hwc, in0=xt[:, cch, :],
                    scalar1=nmu[:, cch:cch + 1], scalar2=ee[:, cch:cch + 1],
                    op0=OP.add, op1=OP.mult)
            hbT = px.tile([128, 4, TOK], bf16, tag="hbT", bufs=2,
                          name=f"hbT{t}")
            for cch in range(NCH):
                nc.sync.dma_start_transpose(
                    out=hbT[:, :, cch * 128:(cch + 1) * 128],
                    in_=hb[:, cch, :])
            return hbT

        def stage_mid_a0(t, xt, hbT):
            # --- A0 = a_norm @ h, 4 replicas at partitions {0,32,64,96};
            #     rows 32r+16 get the constant 1.0 that routes biasu through
            #     the expand matmul (rank-1 accumulation) -------------------
            a0p = ps_small.tile([128, TOK], f32, tag="small")
            for s in range(4):
                rhs8 = hbT[:, s, :].bitcast(fp8).rearrange(
                    "p (n i) -> p i n", i=2)
                nc.tensor.matmul(a0p, lhsT=agt[:, s, :, :], rhs=rhs8,
                                 start=(s == 0), stop=False, perf_mode=DR)
            nc.tensor.matmul(a0p, lhsT=bsel, rhs=ones512,
                             start=False, stop=True)
            a0 = psm.tile([128, TOK], bf16, tag="a0", bufs=2)
            nc.scalar.copy(out=a0, in_=a0p)
            return a0

        def stage_mid(t, xt, hbT, a0):
            # --- expand (4-way row-packed, biasu folded via const row);
            #     relu lands bf16 in SBUF, square runs on the DVE 2x path ----
            rb = px.tile([128, KD, TOK], bf16, tag="rb", bufs=2)
            ubig = px.tile([128, KD, TOK], bf16, tag="ubig", bufs=2)
            for kg in range(2):
                ups = []
                for r in range(4):
                    k = 4 * kg + r
                    up = ps_exp.tile([128, TOK], f32, tag="exp")
                    nc.tensor.matmul(
                        up, lhsT=w1e[32 * r:32 * r + A + 1, k, :],
                        rhs=a0[32 * r:32 * r + A + 1, :],
                        start=True, stop=True,
                        tile_position=(32 * r, 0))
                    ups.append(up)
                for r in range(4):
                    k = 4 * kg + r
                    if k in (3, 7):
                        nc.vector.tensor_scalar(
                            out=rb[:, k, :], in0=ups[r], scalar1=0.0,
                            scalar2=None, op0=OP.max)
                    else:
                        nc.scalar.activation(rb[:, k, :], ups[r], AF.Relu)
                    eng = nc.vector if k in (6, 7) else nc.gpsimd
                    eng.tensor_mul(ubig[:, k, :], rb[:, k, :], rb[:, k, :])

            # --- comp matmul; yb (biased, fp8) + sqy = (yp+b2f)^2 ----------
            yb = px.tile([128, 4, TOK], fp8, tag="yb", bufs=4, name=f"yb{t}")
            sqy = px.tile([128, 4, TOK], bf16, tag="sqy", bufs=3, name=f"sqy{t}")
            for j in range(4):
                yp = ps_y.tile([128, TOK], f32, tag="ypre")
                nc.tensor.matmul(yp[0:64, :], lhsT=w2c[:, 2 * j, :],
                                 rhs=ubig[:, 2 * j, :], start=True, stop=True)
                nc.tensor.matmul(yp[64:128, :], lhsT=w2c[:, 2 * j + 1, :],
                                 rhs=ubig[:, 2 * j + 1, :], start=True,
                                 stop=True, tile_position=(0, 64))
                nc.scalar.activation(yb[:, j, :], yp, AF.Identity,
                                     bias=b2f[:, j:j + 1], scale=1.0)
                nc.scalar.activation(sqy[:, j, :], yp, AF.Square,
                                     bias=b2f[:, j:j + 1], scale=1.0)

            return xt, yb, sqy

        def stage_var(t, sqy):
            # --- per-comp variance matmul; rstd = 1/sqrt(var+eps) ----------
            vst = ps_small.tile([C, TOK], f32, tag="small")
            for j in range(4):
                nc.tensor.matmul(vst, lhsT=vstl[:, j, :], rhs=sqy[:, j, :],
                                 start=(j == 0), stop=(j == 3))
            sd2 = psm.tile([C, TOK], f32, tag="sd2", bufs=2)
            nc.scalar.activation(sd2, vst, AF.Sqrt, bias=cepsp, scale=1.0)
            rr = psm.tile([C, TOK], f32, tag="rr", bufs=2)
            nc.vector.reciprocal_approx_fast(out=rr, in_=sd2)
            rrb = psm.tile([C, TOK], bf16, tag="rrb", bufs=3, name=f"rrb{t}")
            nc.vector.tensor_copy(out=rrb, in_=rr)
            return rrb

        def stage_back(t, xt, yb, rrb):
            row0 = t * TOK
            # rstd broadcast via selector matmuls; ycT = yb * rstd (fp8)
            ycT = px.tile([128, 4, TOK], fp8, tag="ycT", bufs=2)
            for j in range(4):
                rbP = ps_mm.tile([128, TOK], f32, tag="mmout")
                nc.tensor.matmul(rbP, lhsT=sel[:, j, :], rhs=rrb,
                                 start=True, stop=True)
                nc.vector.tensor_mul(ycT[:, j, :], yb[:, j, :], rbP)

            # --- proj (fp8 DoubleRow, operand-swapped) + residual ----------
            upd = px.tile([128, NCH, 2, 512], bf16, tag="upd", bufs=1,
                          name=f"upd{t}")
            for cch in range(NCH):
                osb = px.tile([128, D], f32, tag="osb", bufs=3,
                              name=f"osb{t}_{cch}")
                ud = [ps_mm.tile([128, 512], f32, tag="mmout",
                                 name=f"ud{t}_{cch}_{i}") for i in range(2)]
                for m in range(2):
                    for hf in range(2):
                        nc.tensor.matmul(
                            ud[hf],
                            lhsT=ycT[:, 2 * m:2 * m + 2,
                                     cch * 128:(cch + 1) * 128],
                            rhs=wpf[:, m, :, hf, :],
                            start=(m == 0),
                            stop=(m == 1 and not use_const),
                            perf_mode=DR)
                if use_const:
                    for hf in range(2):
                        nc.tensor.matmul(ud[hf], lhsT=ones1,
                                         rhs=cvec[:, hf, :],
                                         start=False, stop=True)
                for hf in range(2):
                    idx = cch * 2 + hf
                    dst = osb[:, hf * 512:(hf + 1) * 512]
                    xs = xt[:, cch, hf * 512:(hf + 1) * 512]
                    if idx in (1, 3, 4, 6):
                        # Act materializes update (bf16), Pool adds residual
                        nc.scalar.activation(upd[:, cch, hf, :], ud[hf],
                                             AF.Identity, scale=1.0 / SP)
                        nc.gpsimd.tensor_tensor(
                            out=dst, in0=upd[:, cch, hf, :], in1=xs,
                            op=OP.add)
                    else:
                        nc.vector.scalar_tensor_tensor(
                            out=dst, in0=ud[hf], scalar=1.0 / SP,
                            in1=xs, op0=OP.mult, op1=OP.add)
                nc.sync.dma_start(
                    out=out_d[row0 + cch * 128: row0 + (cch + 1) * 128, :],
                    in_=osb)

        fr = {}
        md = {}
        vr = {}
        for t in range(ntile + 3):
            if t < ntile:
                xtf = stage_front_load(t)
                fr[t] = (xtf, stage_front_stats(t, xtf))
            if 2 <= t <= ntile + 1:
                xtm, ybm, sqym = md.pop(t - 2)
                rrb_ = stage_var(t - 2, sqym)
                vr[t - 2] = (xtm, ybm, rrb_)
            if 1 <= t <= ntile:
                xt_, hbT_ = fr.pop(t - 1)
                a0_ = stage_mid_a0(t - 1, xt_, hbT_)
            if t >= 3:
                xtb, ybb, rrbb = vr.pop(t - 3)
                stage_back(t - 3, xtb, ybb, rrbb)
            if 1 <= t <= ntile:
                md[t - 1] = stage_mid(t - 1, xt_, hbT_, a0_)

    nc.compile()
    return nc


def _pack_params(anchors, ln_g, W1, b1, W2, b2, cg, cb, Wp, bp, gate):
    import ml_dtypes
    f32 = np.float32
    bf16 = ml_dtypes.bfloat16
    fp8 = ml_dtypes.float8_e4m3

    anchors = anchors.astype(f32)
    an = anchors / np.maximum(
        np.linalg.norm(anchors.astype(np.float64), axis=1, keepdims=True),
        1e-12).astype(f32)
    ag = (an * ln_g[None, :].astype(f32)).astype(f32)  # [A, D]

    # agt[p, s, i, 32r+m] = 16*ag[m, 4p+s+512i] for r in 0..3 (4 replicas);
    # transposed 16-bit word w=(4p+s) holds the fp8 pair (h[w], h[w+512])
    agt = np.zeros((128, 4, 2, 128), f32)
    ww = np.arange(512)
    pidx, sidx = ww // 4, ww % 4
    for i in range(2):
        for r in range(4):
            agt[pidx, sidx, i, 32 * r:32 * r + A] = 16.0 * ag.T[ww + 512 * i]

    # W1exp[m, f] with m=j*C+k2, f=k*128+e -> value W1[k, j, e] iff k2==k
    W1 = W1.astype(f32)
    w1exp = np.zeros((A, C, E2), f32)
    for m in range(A):
        j, k2 = m // C, m % C
        w1exp[m, k2, :] = W1[k2, j, :]
    sf = w1exp.sum(axis=0)  # [C, E2]
    biasu = sf + b1.astype(f32)  # [C, E2]
    # w1e rows 0..15 = -W1exp (replicated 4x); row 16 = biasu (rides the
    # constant-1 row of a0)
    # anchor rows see a0 = 256*(h.a); bias row rides the exact 1.0 const
    w1e = np.zeros((128, C, E2), f32)
    for r in range(4):
        w1e[32 * r:32 * r + A] = -w1exp / 256.0
        w1e[32 * r + A] = biasu

    W2 = W2.astype(f32)
    w2m = W2.mean(axis=2, keepdims=True)
    w2cent = W2 - w2m  # [C, E2, DC]
    w2c = np.transpose(w2cent, (1, 0, 2)).copy()  # [128, C, 64]
    b2c = b2.astype(f32) - b2.astype(f32).mean(axis=1, keepdims=True)  # [C, DC]

    b2f = np.zeros((128, 4), f32)
    for j in range(4):
        for p in range(128):
            kk = 2 * j + p // 64
            b2f[p, j] = b2c[kk, p % 64]

    # vstl[p, j, c] = 1/64 iff c == 2*j + p//64
    vstl = np.zeros((128, 4, C), f32)
    for j in range(4):
        for p in range(128):
            vstl[p, j, 2 * j + p // 64] = 1.0 / DC

    sig = (1.0 / (1.0 + np.exp(-gate.astype(np.float64)))).astype(f32)  # [D]
    wpfold = (cg.astype(f32).reshape(C * DC, 1) * Wp.astype(f32)) * sig[None, :]
    # wpf[p, m, i, hf, n] = SP * wpfold[(2m+i)*128+p, hf*512+n]
    wpf = np.ascontiguousarray(
        (SP * wpfold).reshape(2, 2, 128, 2, 512).transpose(2, 0, 1, 3, 4))

    const = (cb.astype(f32).reshape(-1) @ Wp.astype(f32) + bp.astype(f32)) * sig
    use_const = bool(np.max(np.abs(const)) > 0)

    sel = np.zeros((C, 4, 128), f32)
    for j in range(4):
        sel[2 * j, j, 0:64] = 1.0
        sel[2 * j + 1, j, 64:128] = 1.0

    params = dict(
        sel=sel.astype(bf16),
        agt=agt.astype(fp8),
        w1e=w1e.astype(bf16),
        w2c=w2c.astype(bf16),
        vstl=vstl.astype(bf16),
        b2f=b2f.astype(f32),
        wpf=wpf.astype(fp8),
    )
    if use_const:
        params["cvec"] = (SP * const).reshape(1, 2, 512).astype(bf16)
    return params, use_const


def kernel(**inputs):
    x = np.asarray(inputs["x"], dtype=np.float32)
    ln_g = np.asarray(inputs["ln_g"], dtype=np.float32)
    ln_b = np.asarray(inputs["ln_b"], dtype=np.float32)

    fast = (np.allclose(ln_g, 1.0, atol=1e-12) and
            np.allclose(ln_b, 0.0, atol=1e-12))
    if not fast:
        return _np_reference(
            x, *[np.asarray(inputs[k], dtype=np.float32) for k in
                 ("anchors", "ln_g", "ln_b", "W1", "b1", "W2", "b2", "cg",
                  "cb", "Wp", "bp", "gate")])

    params, use_const = _pack_params(
        inputs["anchors"], ln_g, inputs["W1"], inputs["b1"], inputs["W2"],
        inputs["b2"], inputs["cg"], inputs["cb"], inputs["Wp"], inputs["bp"],
        inputs["gate"])

    nc = _build_program(S, use_const)

    from concourse.bass_utils import run_bass_kernel_spmd
    in_maps = []
    for b in range(NCORES):
        m = dict(params)
        m["x"] = np.ascontiguousarray(x[b])
        in_maps.append(m)
    res = run_bass_kernel_spmd(nc, in_maps, core_ids=list(range(NCORES)))
    out = np.stack([res.results[b]["out"] for b in range(NCORES)], axis=0)
    return out.reshape(B, S, D).astype(np.float32)


# revision 34
# speedup vs baseline: 1.1033x; 1.1033x over previous
"""Trainium2 Bass kernel for nn_ConstellationRelay.

Computation (per token, D=1024, A=16 anchors, C=8 comps, dc=64):
  h   = l2norm(layernorm(x; ln_g, ln_b))
  tri = 1 - h @ l2norm(anchors).T                       (N, 16)
  u   = relu(einsum('nak,kae->nke', tri_g, W1) + b1)^2  (N, 8, 128)
  y   = layernorm_c(u @ W2 + b2; cg, cb)                (N, 8, 64)
  out = x + sigmoid(gate) * (y.flat @ Wp + bp)

Strategy: pure data-parallel over batch (one of 8 NeuronCores per batch row).
On-device fast path requires ln_g==1, ln_b==0 (always true for this problem's
setup_inputs); every other parameter is handled generally via host-side
folding:
  * h = (x - mu)/sqrt(1024*var)  -- eps cancels exactly through the l2norm
  * tri/W1 stage folded into two small matmuls; biasu (sum_m W1exp + b1) is
    folded into the expand matmul via a constant-1 row in a0, so squared-ReLU
    is a single (max 0) * self op
  * comp-LN mean-subtraction folded into centered W2/b2 (host)
  * cg, cb, bp, sigmoid(gate) folded into Wp/const (host); Wp is fp8 with a
    power-of-2 scale SP compensated in the residual add
  * proj and variance matmuls run in fp8 DoubleRow mode (2 k-chunks/instr)
Layout: token-major for stats/residual, feature-major (via DMA-transpose of
bf16 h) for all matmuls; proj matmul operand-swapped so the residual add
lands token-major in PSUM.
"""

import functools
import os
import sys

import numpy as np

for _p in ("/opt/trn_rl_repo",):
    if _p not in sys.path and os.path.isdir(_p):
        sys.path.insert(0, _p)

B, S, D = 8, 4096, 1024
A, C, DC = 16, 8, 64
APC = A // C  # anchors per compartment
E2 = 2 * DC  # 128, expanded width per comp
NCORES = 8
TOK = 256  # tokens per pipeline tile
NTILE = S // TOK  # 8
NCH = TOK // 128  # 4 token chunks of 128 per tile
KD = D // 128  # 8 feature chunks
SP = 256.0  # fp8 scale on the folded projection matrix


def _np_reference(x, anchors, ln_g, ln_b, W1, b1, W2, b2, cg, cb, Wp, bp, gate):
    """Pure-numpy fallback, mirrors reference.py (used only if ln_g/ln_b
    deviate from the values this problem's setup_inputs produces)."""
    x = x.astype(np.float32)
    N = x.shape[0] * x.shape[1]
    xf = x.reshape(N, D)
    mu = xf.mean(-1, keepdims=True)
    var = ((xf - mu) ** 2).mean(-1, keepdims=True)
    h = (xf - mu) / np.sqrt(var + 1e-5) * ln_g + ln_b
    h = h / np.maximum(np.linalg.norm(h, axis=-1, keepdims=True), 1e-12)
    a = anchors / np.maximum(np.linalg.norm(anchors, axis=-1, keepdims=True), 1e-12)
    tri = 1.0 - h @ a.T
    g = tri.reshape(N, APC, C)
    u = np.einsum("nak,kae->nke", g, W1) + b1
    u = np.square(np.maximum(u, 0.0))
    y = np.einsum("nke,ked->nkd", u, W2) + b2
    muy = y.mean(-1, keepdims=True)
    vy = ((y - muy) ** 2).mean(-1, keepdims=True)
    y = (y - muy) / np.sqrt(vy + 1e-5) * cg + cb
    upd = y.reshape(N, C * DC) @ Wp + bp
    sig = 1.0 / (1.0 + np.exp(-gate))
    return (xf + sig * upd).reshape(x.shape).astype(np.float32)


@functools.lru_cache(maxsize=4)
def _build_program(n_tokens=S, use_const=False):
    """Build + schedule the single-core Bass program (same program runs SPMD
    on all 8 cores)."""
    import concourse.bacc as bacc
    import concourse.mybir as mybir
    import concourse.tile as tile

    f32 = mybir.dt.float32
    bf16 = mybir.dt.bfloat16
    fp8 = mybir.dt.float8e4
    AF = mybir.ActivationFunctionType
    OP = mybir.AluOpType
    DR = mybir.MatmulPerfMode.DoubleRow

    ntile = n_tokens // TOK

    nc = bacc.Bacc("TRN2", target_bir_lowering=False, debug=False,
                   num_devices=NCORES)

    x_d = nc.dram_tensor("x", [n_tokens, D], f32, kind="ExternalInput")
    agt_d = nc.dram_tensor("agt", [128, 4, 2, 128], fp8, kind="ExternalInput")
    w1e_d = nc.dram_tensor("w1e", [128, KD, 128], bf16, kind="ExternalInput")
    w2c_d = nc.dram_tensor("w2c", [128, C, DC], bf16, kind="ExternalInput")
    vstl_d = nc.dram_tensor("vstl", [128, 4, C], bf16, kind="ExternalInput")
    b2f_d = nc.dram_tensor("b2f", [128, 4], f32, kind="ExternalInput")
    wpf_d = nc.dram_tensor("wpf", [128, 2, 2, 2, 512], fp8,
                           kind="ExternalInput")
    sel_d = nc.dram_tensor("sel", [C, 4, 128], bf16, kind="ExternalInput")
    cvec_d = nc.dram_tensor("cvec", [1, 2, 512], bf16, kind="ExternalInput") \
        if use_const else None
    out_d = nc.dram_tensor("out", [n_tokens, D], f32, kind="ExternalOutput")

    from contextlib import ExitStack

    with tile.TileContext(nc) as tc, ExitStack() as ctx:
        ctx.enter_context(nc.allow_low_precision(
            reason="update path is damped by sigmoid(gate)~0.047; fp8/bf16 "
                   "intermediates are well within the 2e-2 tolerance"))
        pp = ctx.enter_context(tc.tile_pool(name="params", bufs=1))
        agt = pp.tile([128, 4, 2, 128], fp8)
        nc.sync.dma_start(out=agt, in_=agt_d[:, :, :, :])
        w1e = pp.tile([128, KD, 128], bf16)
        nc.sync.dma_start(out=w1e, in_=w1e_d[:, :, :])
        w2c = pp.tile([128, C, DC], bf16)
        nc.sync.dma_start(out=w2c, in_=w2c_d[:, :, :])
        vstl = pp.tile([128, 4, C], bf16)
        nc.sync.dma_start(out=vstl, in_=vstl_d[:, :, :])
        b2f = pp.tile([128, 4], f32)
        nc.sync.dma_start(out=b2f, in_=b2f_d[:, :])
        wpf = pp.tile([128, 2, 2, 2, 512], fp8)
        nc.sync.dma_start(out=wpf, in_=wpf_d[:, :, :, :, :])
        sel = pp.tile([C, 4, 128], bf16)
        nc.sync.dma_start(out=sel, in_=sel_d[:, :, :])
        if use_const:
            cvec = pp.tile([1, 2, 512], bf16)
            nc.sync.dma_start(out=cvec, in_=cvec_d[:, :, :])
            ones1 = pp.tile([1, 128], bf16)
            nc.vector.memset(ones1, 1.0)
        ctiny = pp.tile([128, 1], f32)
        nc.vector.memset(ctiny, 1e-38)
        cepsp = pp.tile([C, 1], f32)
        nc.vector.memset(cepsp, 1e-5)
        # Constant-1 routing for biasu: a0p[32r+16, :] = 1.0 via a rank-1
        # accumulation appended to the A0 matmul.
        ones512 = pp.tile([1, TOK], bf16)
        nc.vector.memset(ones512, 1.0)
        bsel = pp.tile([1, 128], bf16)
        nc.vector.memset(bsel, 0.0)
        for r in range(4):
            nc.vector.memset(bsel[0:1, 32 * r + A:32 * r + A + 1], 1.0)

        px = ctx.enter_context(tc.tile_pool(name="px", bufs=2))
        psm = ctx.enter_context(tc.tile_pool(name="psm", bufs=8))
        # PSUM pools: 3 + 2 + 2 + 1 = 8 banks exactly.
        ps_exp = ctx.enter_context(tc.tile_pool(name="ps_exp", bufs=3,
                                                space="PSUM"))
        ps_y = ctx.enter_context(tc.tile_pool(name="ps_y", bufs=2,
                                              space="PSUM"))
        ps_mm = ctx.enter_context(tc.tile_pool(name="ps_mm", bufs=2,
                                               space="PSUM"))
        ps_small = ctx.enter_context(tc.tile_pool(name="ps_small", bufs=1,
                                                  space="PSUM"))

        def stage_front_load(t):
            row0 = t * TOK
            xt = px.tile([128, NCH, D], f32, tag="xt", bufs=4, name=f"xt{t}")
            nc.sync.dma_start(
                out=xt,
                in_=x_d[row0: row0 + TOK, :].rearrange(
                    "(c p) d -> p c d", p=128))
            return xt

        def stage_front_stats(t, xt):
            """Stats + normalize + transpose."""
            hb = px.tile([128, NCH, 512], bf16, tag="hb", bufs=2,
                         name=f"hb{t}")
            mv = psm.tile([128, NCH, 2], f32, tag="mv", name=f"mv{t}")
            for cch in range(NCH):
                st = psm.tile([128, 2, 6], f32, tag="st")
                xr = xt[:, cch, :].rearrange("p (s f) -> p s f", s=2)
                nc.vector.bn_stats(out=st[:, 0, :], in_=xr[:, 0, :])
                nc.vector.bn_stats(out=st[:, 1, :], in_=xr[:, 1, :])
                nc.vector.bn_aggr(out=mv[:, cch, :], in_=st)
            # ee = 16/sqrt(D*var + tiny) = 16/||x-mu|| (fp8 h scaled by 16)
            sd = psm.tile([128, NCH], f32, tag="sd")
            nc.scalar.activation(sd, mv[:, :, 1], AF.Sqrt, bias=ctiny,
                                 scale=float(D) / 256.0)
            ee = psm.tile([128, NCH], f32, tag="ee", name=f"ee{t}")
            nc.vector.reciprocal(ee, sd)
            bh = psm.tile([128, NCH], f32, tag="bh", name=f"bh{t}")
            nc.vector.scalar_tensor_tensor(
                out=bh, in0=mv[:, :, 0], scalar=-1.0, in1=ee,
                op0=OP.mult, op1=OP.mult)
            nmu = psm.tile([128, NCH], f32, tag="nmu", name=f"nmu{t}")
            nc.vector.tensor_scalar_mul(nmu, mv[:, :, 0], -1.0)
            # hb word w packs fp8 pair (16*h[w], 16*h[w+512]); the Act/Pool
            # out AP iterates the pair halves as the outer free dim
            # (chunk 0 on Act, chunks 1-3 on Pool)
            for cch in range(2):
                hwa = hb[:, cch, :].bitcast(fp8).rearrange(
                    "p (w i) -> p i w", i=2)
                nc.scalar.activation(hwa, xt[:, cch, :], AF.Identity,
                                     bias=bh[:, cch:cch + 1],
                                     scale=ee[:, cch:cch + 1])
            for cch in range(2, NCH):
                hwc = hb[:, cch, :].bitcast(fp8).rearrange(
                    "p (w i) -> p i w", i=2)
                nc.gpsimd.tensor_scalar(
                    out=hwc, in0=xt[:, cch, :],
                    scalar1=nmu[:, cch:cch + 1], scalar2=ee[:, cch:cch + 1],
                    op0=OP.add, op1=OP.mult)
            hbT = px.tile([128, 4, TOK], bf16, tag="hbT", bufs=2,
                          name=f"hbT{t}")
            for cch in range(NCH):
                nc.sync.dma_start_transpose(
                    out=hbT[:, :, cch * 128:(cch + 1) * 128],
                    in_=hb[:, cch, :])
            return hbT

        def stage_mid_a0(t, xt, hbT):
            # --- A0 = a_norm @ h, 4 replicas at partitions {0,32,64,96};
            #     rows 32r+16 get the constant 1.0 that routes biasu through
            #     the expand matmul (rank-1 accumulation) -------------------
            a0p = ps_small.tile([128, TOK], f32, tag="small")
            for s in range(4):
                rhs8 = hbT[:, s, :].bitcast(fp8).rearrange(
                    "p (n i) -> p i n", i=2)
                nc.tensor.matmul(a0p, lhsT=agt[:, s, :, :], rhs=rhs8,
                                 start=(s == 0), stop=False, perf_mode=DR)
            nc.tensor.matmul(a0p, lhsT=bsel, rhs=ones512,
                             start=False, stop=True)
            a0 = psm.tile([128, TOK], bf16, tag="a0", bufs=2)
            nc.scalar.copy(out=a0, in_=a0p)
            return a0

        def stage_mid(t, xt, hbT, a0):
            # --- expand (4-way row-packed, biasu folded via const row);
            #     relu lands bf16 in SBUF, square runs on the DVE 2x path ----
            rb = px.tile([128, KD, TOK], bf16, tag="rb", bufs=2)
            ubig = px.tile([128, KD, TOK], bf16, tag="ubig", bufs=2)
            for kg in range(2):
                ups = []
                for r in range(4):
                    k = 4 * kg + r
                    up = ps_exp.tile([128, TOK], f32, tag="exp")
                    nc.tensor.matmul(
                        up, lhsT=w1e[32 * r:32 * r + A + 1, k, :],
                        rhs=a0[32 * r:32 * r + A + 1, :],
                        start=True, stop=True,
                        tile_position=(32 * r, 0))
                    ups.append(up)
                for r in range(4):
                    k = 4 * kg + r
                    nc.scalar.activation(rb[:, k, :], ups[r], AF.Relu)
                    nc.gpsimd.tensor_mul(ubig[:, k, :], rb[:, k, :],
                                         rb[:, k, :])

            # --- comp matmul; yb (biased, fp8) + sqy = (yp+b2f)^2 ----------
            yb = px.tile([128, 4, TOK], fp8, tag="yb", bufs=4, name=f"yb{t}")
            sqy = px.tile([128, 4, TOK], bf16, tag="sqy", bufs=3, name=f"sqy{t}")
            for j in range(4):
                yp = ps_y.tile([128, TOK], f32, tag="ypre")
                nc.tensor.matmul(yp[0:64, :], lhsT=w2c[:, 2 * j, :],
                                 rhs=ubig[:, 2 * j, :], start=True, stop=True)
                nc.tensor.matmul(yp[64:128, :], lhsT=w2c[:, 2 * j + 1, :],
                                 rhs=ubig[:, 2 * j + 1, :], start=True,
                                 stop=True, tile_position=(0, 64))
                nc.scalar.activation(yb[:, j, :], yp, AF.Identity,
                                     bias=b2f[:, j:j + 1], scale=1.0)
                nc.scalar.activation(sqy[:, j, :], yp, AF.Square,
                                     bias=b2f[:, j:j + 1], scale=1.0)

            return xt, yb, sqy

        def stage_var(t, sqy):
            # --- per-comp variance matmul; rstd = 1/sqrt(var+eps) ----------
            vst = ps_small.tile([C, TOK], f32, tag="small")
            for j in range(4):
                nc.tensor.matmul(vst, lhsT=vstl[:, j, :], rhs=sqy[:, j, :],
                                 start=(j == 0), stop=(j == 3))
            sd2 = psm.tile([C, TOK], f32, tag="sd2", bufs=2)
            nc.scalar.activation(sd2, vst, AF.Sqrt, bias=cepsp, scale=1.0)
            rr = psm.tile([C, TOK], f32, tag="rr", bufs=2)
            nc.vector.reciprocal_approx_fast(out=rr, in_=sd2)
            rrb = psm.tile([C, TOK], bf16, tag="rrb", bufs=3, name=f"rrb{t}")
            nc.vector.tensor_copy(out=rrb, in_=rr)
            return rrb

        def stage_back(t, xt, yb, rrb):
            row0 = t * TOK
            # rstd broadcast via selector matmuls; ycT = yb * rstd (fp8)
            ycT = px.tile([128, 4, TOK], fp8, tag="ycT", bufs=2)
            for j in range(4):
                rbP = ps_mm.tile([128, TOK], f32, tag="mmout")
                nc.tensor.matmul(rbP, lhsT=sel[:, j, :], rhs=rrb,
                                 start=True, stop=True)
                nc.vector.tensor_mul(ycT[:, j, :], yb[:, j, :], rbP)

            # --- proj (fp8 DoubleRow, operand-swapped) + residual ----------
            upd = px.tile([128, NCH, 2, 512], bf16, tag="upd", bufs=1,
                          name=f"upd{t}")
            for cch in range(NCH):
                osb = px.tile([128, D], f32, tag="osb", bufs=3,
                              name=f"osb{t}_{cch}")
                ud = [ps_mm.tile([128, 512], f32, tag="mmout",
                                 name=f"ud{t}_{cch}_{i}") for i in range(2)]
                for m in range(2):
                    for hf in range(2):
                        nc.tensor.matmul(
                            ud[hf],
                            lhsT=ycT[:, 2 * m:2 * m + 2,
                                     cch * 128:(cch + 1) * 128],
                            rhs=wpf[:, m, :, hf, :],
                            start=(m == 0),
                            stop=(m == 1 and not use_const),
                            perf_mode=DR)
                if use_const:
                    for hf in range(2):
                        nc.tensor.matmul(ud[hf], lhsT=ones1,
                                         rhs=cvec[:, hf, :],
                                         start=False, stop=True)
                for hf in range(2):
                    idx = cch * 2 + hf
                    dst = osb[:, hf * 512:(hf + 1) * 512]
                    xs = xt[:, cch, hf * 512:(hf + 1) * 512]
                    if idx in (3, 6):
                        # Act materializes update (bf16), Pool adds residual
                        nc.scalar.activation(upd[:, cch, hf, :], ud[hf],
                                             AF.Identity, scale=1.0 / SP)
                        nc.gpsimd.tensor_tensor(
                            out=dst, in0=upd[:, cch, hf, :], in1=xs,
                            op=OP.add)
                    else:
                        nc.vector.scalar_tensor_tensor(
                            out=dst, in0=ud[hf], scalar=1.0 / SP,
                            in1=xs, op0=OP.mult, op1=OP.add)
                nc.sync.dma_start(
                    out=out_d[row0 + cch * 128: row0 + (cch + 1) * 128, :],
                    in_=osb)

        ld = {}
        fr = {}
        md = {}
        vr = {}
        for t in range(ntile + 3):
            if t < ntile:
                ld[t] = stage_front_load(t)
            if 2 <= t <= ntile + 1:
                xtm, ybm, sqym = md.pop(t - 2)
                rrb_ = stage_var(t - 2, sqym)
                vr[t - 2] = (xtm, ybm, rrb_)
            if 1 <= t <= ntile:
                xt_, hbT_ = fr.pop(t - 1)
                a0_ = stage_mid_a0(t - 1, xt_, hbT_)
            if t >= 3:
                xtb, ybb, rrbb = vr.pop(t - 3)
                stage_back(t - 3, xtb, ybb, rrbb)
            if t < ntile:
                xtf = ld.pop(t)
                fr[t] = (xtf, stage_front_stats(t, xtf))
            if 1 <= t <= ntile:
                md[t - 1] = stage_mid(t - 1, xt_, hbT_, a0_)

    nc.compile()
    return nc


def _pack_params(anchors, ln_g, W1, b1, W2, b2, cg, cb, Wp, bp, gate):
    import ml_dtypes
    f32 = np.float32
    bf16 = ml_dtypes.bfloat16
    fp8 = ml_dtypes.float8_e4m3

    anchors = anchors.astype(f32)
    an = anchors / np.maximum(
        np.linalg.norm(anchors.astype(np.float64), axis=1, keepdims=True),
        1e-12).astype(f32)
    ag = (an * ln_g[None, :].astype(f32)).astype(f32)  # [A, D]

    # agt[p, s, i, 32r+m] = 16*ag[m, 4p+s+512i] for r in 0..3 (4 replicas);
    # transposed 16-bit word w=(4p+s) holds the fp8 pair (h[w], h[w+512])
    agt = np.zeros((128, 4, 2, 128), f32)
    ww = np.arange(512)
    pidx, sidx = ww // 4, ww % 4
    for i in range(2):
        for r in range(4):
            agt[pidx, sidx, i, 32 * r:32 * r + A] = 16.0 * ag.T[ww + 512 * i]

    # W1exp[m, f] with m=j*C+k2, f=k*128+e -> value W1[k, j, e] iff k2==k
    W1 = W1.astype(f32)
    w1exp = np.zeros((A, C, E2), f32)
    for m in range(A):
        j, k2 = m // C, m % C
        w1exp[m, k2, :] = W1[k2, j, :]
    sf = w1exp.sum(axis=0)  # [C, E2]
    biasu = sf + b1.astype(f32)  # [C, E2]
    # w1e rows 0..15 = -W1exp (replicated 4x); row 16 = biasu (rides the
    # constant-1 row of a0)
    # anchor rows see a0 = 256*(h.a); bias row rides the exact 1.0 const
    w1e = np.zeros((128, C, E2), f32)
    for r in range(4):
        w1e[32 * r:32 * r + A] = -w1exp / 256.0
        w1e[32 * r + A] = biasu

    W2 = W2.astype(f32)
    w2m = W2.mean(axis=2, keepdims=True)
    w2cent = W2 - w2m  # [C, E2, DC]
    w2c = np.transpose(w2cent, (1, 0, 2)).copy()  # [128, C, 64]
    b2c = b2.astype(f32) - b2.astype(f32).mean(axis=1, keepdims=True)  # [C, DC]

    b2f = np.zeros((128, 4), f32)
    for j in range(4):
        for p in range(128):
            kk = 2 * j + p // 64
            b2f[p, j] = b2c[kk, p % 64]

    # vstl[p, j, c] = 1/64 iff c == 2*j + p//64
    vstl = np.zeros((128, 4, C), f32)
    for j in range(4):
        for p in range(128):
            vstl[p, j, 2 * j + p // 64] = 1.0 / DC

    sig = (1.0 / (1.0 + np.exp(-gate.astype(np.float64)))).astype(f32)  # [D]
    wpfold = (cg.astype(f32).reshape(C * DC, 1) * Wp.astype(f32)) * sig[None, :]
    # wpf[p, m, i, hf, n] = SP * wpfold[(2m+i)*128+p, hf*512+n]
    wpf = np.ascontiguousarray(
        (SP * wpfold).reshape(2, 2, 128, 2, 512).transpose(2, 0, 1, 3, 4))

    const = (cb.astype(f32).reshape(-1) @ Wp.astype(f32) + bp.astype(f32)) * sig
    use_const = bool(np.max(np.abs(const)) > 0)

    sel = np.zeros((C, 4, 128), f32)
    for j in range(4):
        sel[2 * j, j, 0:64] = 1.0
        sel[2 * j + 1, j, 64:128] = 1.0

    params = dict(
        sel=sel.astype(bf16),
        agt=agt.astype(fp8),
        w1e=w1e.astype(bf16),
        w2c=w2c.astype(bf16),
        vstl=vstl.astype(bf16),
        b2f=b2f.astype(f32),
        wpf=wpf.astype(fp8),
    )
    if use_const:
        params["cvec"] = (SP * const).reshape(1, 2, 512).astype(bf16)
    return params, use_const


def kernel(**inputs):
    x = np.asarray(inputs["x"], dtype=np.float32)
    ln_g = np.asarray(inputs["ln_g"], dtype=np.float32)
    ln_b = np.asarray(inputs["ln_b"], dtype=np.float32)

    fast = (np.allclose(ln_g, 1.0, atol=1e-12) and
            np.allclose(ln_b, 0.0, atol=1e-12))
    if not fast:
        return _np_reference(
            x, *[np.asarray(inputs[k], dtype=np.float32) for k in
                 ("anchors", "ln_g", "ln_b", "W1", "b1", "W2", "b2", "cg",
                  "cb", "Wp", "bp", "gate")])

    params, use_const = _pack_params(
        inputs["anchors"], ln_g, inputs["W1"], inputs["b1"], inputs["W2"],
        inputs["b2"], inputs["cg"], inputs["cb"], inputs["Wp"], inputs["bp"],
        inputs["gate"])

    nc = _build_program(S, use_const)

    from concourse.bass_utils import run_bass_kernel_spmd
    in_maps = []
    for b in range(NCORES):
        m = dict(params)
        m["x"] = np.ascontiguousarray(x[b])
        in_maps.append(m)
    res = run_bass_kernel_spmd(nc, in_maps, core_ids=list(range(NCORES)))
    out = np.stack([res.results[b]["out"] for b in range(NCORES)], axis=0)
    return out.reshape(B, S, D).astype(np.float32)
